# revision 58
# baseline (speedup 1.0000x reference)
"""CantorAttention Trainium2 kernel (8 NeuronCores, SPMD).

Strategy:
  - Shard batch (2) x head-groups (4 heads each) across the 8 cores.
  - Host: sort sequence positions by their Cantor value. Route rows depend
    only on the position's Cantor value, so after this permutation every
    128-query block attends to a narrow contiguous band of keys (~256).
  - Device per core: QKV projection (bf16 matmuls), banded masked attention
    (scores computed transposed so the attention output lands directly in
    the [dim, seq] layout the output projection needs), and the per-core
    partial output projection.
  - Host: sum the 4 per-batch partials, add b_out, un-permute rows.

Correct for arbitrary routes tables: bands/masks are derived from the actual
routes input; the Cantor sort is only a (data-independent) heuristic that
makes the bands tight for Cantor-routed inputs.
"""

import os
import sys

sys.path.insert(0, "/opt/trn_rl_repo")

import numpy as np
import ml_dtypes

import concourse.bass as bass
import concourse.mybir as mybir
import concourse.tile as tile
from concourse import bacc
from concourse.bass_utils import run_bass_kernel_spmd

B, S, DIM, H, HD, KNN, DEPTH = 2, 2048, 1024, 16, 64, 64, 8
SCALE = 1.0 / np.sqrt(HD)
N_CORES = 8
HPC = H // (N_CORES // B)       # heads per core = 4
FQK = 2 * HPC * HD              # q+k rows per core = 512
BLK = 128                       # queries per attention block
NBLK = S // BLK                 # 16

F32 = mybir.dt.float32
BF16 = mybir.dt.bfloat16
BF16NP = ml_dtypes.bfloat16

LAST_RESULTS = None  # BassKernelResults of the most recent run (for test.py)
_PROGRAM_CACHE = {}


def _ensure_axon_hooks():
    """Provide antenv.axon_hooks if the image lacks it, wiring the NTFF
    profile hook from the boot shim so BASS_TRACE=1 can capture timings."""
    try:
        import antenv.axon_hooks  # noqa: F401
        return
    except ImportError:
        pass
    import types
    import antenv
    hook = None
    try:
        from trn_agent_boot.trn_boot import _ntff_profile_via_ctypes
        if os.path.exists("/opt/axon/libaxon_pjrt.so"):
            hook = _ntff_profile_via_ctypes("/opt/axon/libaxon_pjrt.so")
    except Exception:
        hook = None
    mod = types.ModuleType("antenv.axon_hooks")
    mod.get_axon_ntff_profile_hook = lambda: hook
    mod.set_axon_ntff_profile_hook = lambda h: None
    sys.modules["antenv.axon_hooks"] = mod
    antenv.axon_hooks = mod


def _patch_upload():
    """Don't attempt S3 artifact uploads from the sandbox."""
    import concourse.bass_utils as bu
    bu.upload_artifacts = lambda tmpdir: str(tmpdir)


_ensure_axon_hooks()
_patch_upload()


def _cantor_values(seq_len, depth):
    pos = np.arange(seq_len, dtype=np.float64)
    x = pos / max(1, seq_len - 1)
    x = np.clip(x, 1e-06, 1.0 - 1e-06)
    cantor = np.zeros(seq_len, dtype=np.float64)
    factor = 0.5
    for _ in range(depth):
        x = x * 3.0
        digit = np.floor(x)
        x = x - digit
        cantor += factor * (digit == 2.0)
        factor *= 0.5
    return cantor.astype(np.float32)


def _plan_bands(routes_p):
    """Per 128-query block: (lo, n_subtiles) with 128-multiple band widths."""
    lo_all = routes_p.min(axis=1).reshape(NBLK, BLK).min(axis=1)
    hi_all = (routes_p.max(axis=1) + 1).reshape(NBLK, BLK).max(axis=1)
    bands = []
    for b in range(NBLK):
        lo, hi = int(lo_all[b]), int(hi_all[b])
        lo = (lo // 32) * 32       # engine ops need 32-aligned start partitions
        u = int(np.ceil((hi - lo) / 128.0)) * 128
        u = max(u, 128)
        lo = min(lo, S - u)
        bands.append((lo, u // 128))
    return bands


def _build_masks(routes_p, bands):
    """Count-masks in device layout [128, 2, nU, BLK] bf16 (head-pair dup)."""
    parts = []
    for b, (lo, nb) in enumerate(bands):
        rel = routes_p[b * BLK:(b + 1) * BLK] - lo          # [BLK, KNN]
        m = np.zeros((nb * 128, BLK), dtype=np.float32)
        qidx = np.broadcast_to(np.arange(BLK)[:, None], rel.shape)
        np.add.at(m, (rel, qidx), 1.0)
        parts.append(m)
    mk = np.concatenate(parts, axis=0)                      # [nU*128, BLK]
    nU = mk.shape[0] // 128
    mk = mk.reshape(nU, 128, BLK).transpose(1, 0, 2)        # [128, nU, BLK]
    mk = np.broadcast_to(mk[:, :, None], (128, nU, 2, BLK))
    return np.ascontiguousarray(mk).astype(BF16NP)


def _build_program(bands):
    """Emit the SPMD Bass program for the given band plan."""
    nU = sum(nb for _, nb in bands)
    nb_max = max(nb for _, nb in bands)
    debug = bool(os.environ.get("KM_DEBUG"))

    nc = bacc.Bacc("TRN2", target_bir_lowering=False)

    xT_d = nc.dram_tensor("xT", [DIM, S], BF16, kind="ExternalInput")
    wq_d = nc.dram_tensor("wqkvT", [DIM, FQK + HPC * HD], BF16, kind="ExternalInput")
    bqk_d = nc.dram_tensor("bqk", [FQK], F32, kind="ExternalInput")
    bv_d = nc.dram_tensor("bv", [HPC * HD], F32, kind="ExternalInput")
    wo_d = nc.dram_tensor("woT", [HPC * HD, DIM], BF16, kind="ExternalInput")
    # pre-arranged mask layout [128, nU, 2, BLK] (head-pair duplicated), bf16
    mask_d = nc.dram_tensor("maskT", [128, nU, 2, BLK], BF16, kind="ExternalInput")
    out_d = nc.dram_tensor("out_p", [S, DIM], F32, kind="ExternalOutput")
    if debug:
        dbg_qk = nc.dram_tensor("dbg_qk", [128, FQK // 128, S], BF16, kind="ExternalOutput")
        dbg_v = nc.dram_tensor("dbg_v", [128, S // 128, HPC * HD], BF16, kind="ExternalOutput")
        dbg_stg = nc.dram_tensor("dbg_stg", [128, 2, S], F32, kind="ExternalOutput")
        dbg_den = nc.dram_tensor("dbg_den", [HPC, S], F32, kind="ExternalOutput")
        dbg_attnT = nc.dram_tensor("dbg_attnT", [128, 2, S], BF16, kind="ExternalOutput")
        dbg_pd = nc.dram_tensor("dbg_pd", [NBLK * HPC, BLK], F32, kind="ExternalOutput")

    KT = DIM // 128  # 8 contraction tiles

    with tile.TileContext(nc) as tc:
        with tc.tile_pool(name="const", bufs=1) as cpool, \
             tc.tile_pool(name="work", bufs=1) as wpool, \
             tc.tile_pool(name="epool", bufs=6) as epool, \
             tc.tile_pool(name="spool", bufs=2) as spool, \
             tc.tile_pool(name="dram", bufs=1, space="DRAM") as dpool, \
             tc.tile_pool(name="pp", bufs=2, space="PSUM") as pp, \
             tc.tile_pool(name="ps", bufs=3, space="PSUM") as ps, \
             tc.tile_pool(name="pv", bufs=3, space="PSUM") as pv:

            # ---- constant loads ----
            xT = cpool.tile([128, KT, S], BF16, tag="xT")
            for kt in range(KT):
                nc.sync.dma_start(
                    xT[:, kt, :],
                    xT_d.rearrange("(t p) s -> p t s", p=128)[:, kt, :])
            wq = cpool.tile([128, KT, FQK + HPC * HD], BF16, tag="wq")
            nc.sync.dma_start(wq[:], wq_d.rearrange("(t p) f -> p t f", p=128))
            bqk = cpool.tile([128, FQK // 128], F32, tag="bqk")
            nc.sync.dma_start(bqk[:], bqk_d.rearrange("(t p) -> p t", p=128))
            bvb = cpool.tile([128, HPC * HD], F32, tag="bvb")
            nc.sync.dma_start(bvb[:], bv_d[None, :].to_broadcast((128, HPC * HD)))
            wo = cpool.tile([128, 2, DIM], BF16, tag="wo")
            nc.sync.dma_start(wo[:], wo_d.rearrange("(t p) o -> p t o", p=128))


            # ---- phase A: q/k projection -> qk_sb [128, 4, S] (f-major) ----
            # f-layout rows: [q_h0 q_h1 | q_h2 q_h3 | k_h0 k_h1 | k_h2 k_h3]
            qk_sb = wpool.tile([128, FQK // 128, S], BF16, tag="qk")
            for ft in range(FQK // 128):
                for st in range(S // 512):
                    pt = pp.tile([128, 512], F32, tag="pp")
                    for kt in range(KT):
                        nc.tensor.matmul(
                            pt[:],
                            wq[:, kt, ft * 128:(ft + 1) * 128],
                            xT[:, kt, st * 512:(st + 1) * 512],
                            start=(kt == 0), stop=(kt == KT - 1))
                    nc.scalar.activation(
                        qk_sb[:, ft, st * 512:(st + 1) * 512], pt[:],
                        mybir.ActivationFunctionType.Identity,
                        bias=bqk[:, ft:ft + 1])

            # ---- phase B: v projection -> v_sb [128, 16, HPC, 65] ----
            # (64 v cols per head + a baked ones column for the denominator)
            v_sb = wpool.tile([128, S // 128, HPC, HD + 1], BF16, tag="v")
            nc.vector.memset(v_sb[:], 1.0)
            for st in range(S // 128):
                pt = pp.tile([128, 512], F32, tag="pp")
                for kt in range(KT):
                    nc.tensor.matmul(
                        pt[:, :HPC * HD],
                        xT[:, kt, st * 128:(st + 1) * 128],
                        wq[:, kt, FQK:],
                        start=(kt == 0), stop=(kt == KT - 1))
                nc.vector.tensor_add(
                    v_sb[:, st, :, :HD],
                    pt[:, :HPC * HD].rearrange("p (h d) -> p h d", h=HPC),
                    bvb.rearrange("p (h d) -> p h d", h=HPC))

            # ---- attention ----
            stg_un = wpool.tile([128, 2, S], F32, tag="stg")     # unnormalized attnT
            den_dram = dpool.tile([HPC, S], F32)
            # per-head denominator rows (kept < 8KB free offsets per tile)
            den_sb = [wpool.tile([1, S], F32, tag=f"den{h}", name=f"den_sb{h}")
                      for h in range(HPC)]
            wide = nb_max > 2 or nU > 64

            def pack_band(dst, dsl, b):
                """band-pack v (+ones cols) via DVE cross-base chunk copies."""
                lo, nb = bands[b]
                a0, r = lo // 128, lo % 128
                if r == 0:
                    nc.vector.tensor_copy(dst[:, dsl], v_sb[:, a0:a0 + nb])
                else:
                    for j in range(4):
                        sp = (r + 32 * j) % 128
                        sa = a0 + (1 if r + 32 * j >= 128 else 0)
                        nc.vector.tensor_copy(
                            dst[32 * j:32 * (j + 1), dsl],
                            v_sb[sp:sp + 32, sa:sa + nb])

            if not wide:
                # resident masks (host pre-arranged, contiguous load)
                mk = wpool.tile([128, nU, 2, BLK], BF16, tag="mask")
                nc.sync.dma_start(mk[:], mask_d[:])
                # band-packed V for all blocks: [128, nU, HPC, 65]
                vpk = wpool.tile([128, nU, HPC, HD + 1], BF16, tag="vpk")
                moff = 0
                for b in range(NBLK):
                    pack_band(vpk, slice(moff, moff + bands[b][1]), b)
                    moff += bands[b][1]
            moff = 0
            if wide:
                # general fallback (arbitrary routes): stream masks/V per block
                for b in range(NBLK):
                    lo, nb = bands[b]
                    qs = slice(b * BLK, (b + 1) * BLK)
                    mkb = spool.tile([128, nb_max, 2, BLK], BF16, tag="mkb")
                    nc.sync.dma_start(mkb[:, :nb], mask_d[:, moff:moff + nb])
                    vpb = spool.tile([128, nb_max, HPC, HD + 1], BF16, tag="vpb")
                    pack_band(vpb, slice(0, nb), b)
                    for h in range(HPC):
                        hh, hp = h % 2, h // 2
                        pvt = pv.tile([HD + 1, BLK], F32, tag="pv")
                        for iu in range(nb):
                            pst = ps.tile([128, BLK], F32, tag="ps")
                            nc.tensor.matmul(
                                pst[:],
                                qk_sb[64 * hh:64 * hh + 64, 2 + hp,
                                      lo + iu * 128: lo + (iu + 1) * 128],
                                qk_sb[64 * hh:64 * hh + 64, hp, qs],
                                start=True, stop=True)
                            et = epool.tile([128, BLK], BF16, tag="e")
                            nc.scalar.activation(
                                et[:], pst[:], mybir.ActivationFunctionType.Exp,
                                scale=float(SCALE))
                            emt = epool.tile([128, BLK], BF16, tag="em")
                            nc.vector.tensor_mul(emt[:], et[:],
                                                 mkb[:, iu, hh, :])
                            nc.tensor.matmul(
                                pvt[:], vpb[:, iu, h, :], emt[:],
                                start=(iu == 0), stop=(iu == nb - 1))
                        nc.scalar.copy(stg_un[64 * hh:64 * hh + 64, hp, qs], pvt[:HD, :])
                        nc.vector.reciprocal(den_sb[h][0:1, qs], pvt[HD:HD + 1, :])
                        if debug:
                            nc.sync.dma_start(dbg_pd[b * HPC + h:b * HPC + h + 1, :],
                                              den_sb[h][0:1, qs])
                    moff += nb
            else:
                for b in range(NBLK):
                    lo, nb = bands[b]
                    qs = slice(b * BLK, (b + 1) * BLK)
                    for h in range(HPC):
                        hh, hp = h % 2, h // 2
                        pst = ps.tile([128, nb_max, BLK], F32, tag="ps")
                        for iu in range(nb):
                            nc.tensor.matmul(
                                pst[:, iu, :],
                                qk_sb[64 * hh:64 * hh + 64, 2 + hp,
                                      lo + iu * 128: lo + (iu + 1) * 128],
                                qk_sb[64 * hh:64 * hh + 64, hp, qs],
                                start=True, stop=True)
                        et = epool.tile([128, nb_max, BLK], BF16, tag="e")
                        nc.scalar.activation(
                            et[:, :nb], pst[:, :nb],
                            mybir.ActivationFunctionType.Exp, scale=float(SCALE))
                        emt = epool.tile([128, nb_max, BLK], BF16, tag="em")
                        nc.vector.tensor_mul(emt[:, :nb], et[:, :nb],
                                             mk[:, moff:moff + nb, hh, :])
                        pvt = pv.tile([HD + 1, BLK], F32, tag="pv")
                        for iu in range(nb):
                            nc.tensor.matmul(
                                pvt[:], vpk[:, moff + iu, h, :], emt[:, iu, :],
                                start=(iu == 0), stop=(iu == nb - 1))
                        nc.scalar.copy(stg_un[64 * hh:64 * hh + 64, hp, qs],
                                       pvt[:HD, :])
                        nc.vector.reciprocal(den_sb[h][0:1, qs], pvt[HD:HD + 1, :])
                        if debug:
                            nc.sync.dma_start(
                                dbg_pd[b * HPC + h:b * HPC + h + 1, :],
                                den_sb[h][0:1, qs])
                    moff += nb
            # ---- normalize + output projection, pipelined in s-quarters ----
            # denominators for queries < q0 are final once the blocks covering
            # them are done, so each quarter's normalize/outproj overlaps the
            # remaining attention blocks.
            rec_bc = wpool.tile([128, 2, S], F32, tag="denbc")
            attnT = wpool.tile([128, 2, S], BF16, tag="attnT")
            NQ = 4
            SQ = S // NQ
            for q in range(NQ):
                sq = slice(q * SQ, (q + 1) * SQ)
                for h in range(HPC):
                    nc.sync.dma_start(den_dram[h:h + 1, sq], den_sb[h][0:1, sq])
                for dt in range(2):
                    for hh in range(2):
                        h = 2 * dt + hh
                        nc.sync.dma_start(
                            rec_bc[64 * hh:64 * (hh + 1), dt, sq],
                            den_dram[h:h + 1, sq].to_broadcast((64, SQ)))
                    nc.vector.tensor_mul(
                        attnT[:, dt, sq], stg_un[:, dt, sq], rec_bc[:, dt, sq])
                for st in range(q * (S // 128) // NQ, (q + 1) * (S // 128) // NQ):
                    for ot in range(DIM // 512):
                        po = pp.tile([128, 512], F32, tag="pp")
                        for dt in range(2):
                            nc.tensor.matmul(
                                po[:],
                                attnT[:, dt, st * 128:(st + 1) * 128],
                                wo[:, dt, ot * 512:(ot + 1) * 512],
                                start=(dt == 0), stop=(dt == 1))
                        ob = epool.tile([128, 512], F32, tag="ob")
                        nc.vector.tensor_copy(ob[:], po[:])
                        nc.sync.dma_start(
                            out_d[st * 128:(st + 1) * 128, ot * 512:(ot + 1) * 512],
                            ob[:])

            if debug:
                nc.sync.dma_start(dbg_qk[:], qk_sb[:])
                nc.sync.dma_start(dbg_v[:], v_sb[:])
                nc.sync.dma_start(dbg_stg[:], stg_un[:])
                nc.sync.dma_start(dbg_den[:], den_dram[:])
                nc.sync.dma_start(dbg_attnT[:], attnT[:])

    nc.finalize()
    return nc


def kernel(x, w_qkv, b_qkv, w_out, b_out, routes):
    global LAST_RESULTS
    x = np.asarray(x, dtype=np.float32)
    w_qkv = np.asarray(w_qkv, dtype=np.float32)
    b_qkv = np.asarray(b_qkv, dtype=np.float32)
    w_out = np.asarray(w_out, dtype=np.float32)
    b_out = np.asarray(b_out, dtype=np.float32)
    routes = np.asarray(routes)

    # --- host: permutation + bands + masks ---
    cantor = _cantor_values(S, DEPTH)
    perm = np.lexsort((np.arange(S), cantor))
    inv_perm = np.empty(S, dtype=np.int64)
    inv_perm[perm] = np.arange(S)
    routes_p = inv_perm[routes.astype(np.int64)[perm]]
    bands = _plan_bands(routes_p)
    maskT = _build_masks(routes_p, bands)

    key = (tuple(bands), bool(os.environ.get("KM_DEBUG")))
    if key not in _PROGRAM_CACHE:
        _PROGRAM_CACHE[key] = _build_program(bands)
    nc = _PROGRAM_CACHE[key]

    # --- host: per-core inputs ---
    x_p = x[:, perm, :]                                   # [B, S, DIM]
    in_maps = []
    for c in range(N_CORES):
        b = c // (N_CORES // B)
        hg = c % (N_CORES // B)
        heads = range(hg * HPC, (hg + 1) * HPC)
        # w rows: q heads, k heads, v heads
        rows = ([h * HD + i for h in heads for i in range(HD)]
                + [DIM + h * HD + i for h in heads for i in range(HD)]
                + [2 * DIM + h * HD + i for h in heads for i in range(HD)])
        rows = np.asarray(rows)
        wq_c = np.ascontiguousarray(w_qkv[rows].T).astype(BF16NP)   # [1024, 768]
        bqk_c = np.ascontiguousarray(b_qkv[rows[:FQK]]).astype(np.float32)
        bv_c = np.ascontiguousarray(b_qkv[rows[FQK:]]).astype(np.float32)
        wo_c = np.ascontiguousarray(
            w_out[:, hg * HPC * HD:(hg + 1) * HPC * HD].T).astype(BF16NP)
        in_maps.append({
            "xT": np.ascontiguousarray(x_p[b].T).astype(BF16NP),
            "wqkvT": wq_c,
            "bqk": bqk_c,
            "bv": bv_c,
            "woT": wo_c,
            "maskT": maskT,
        })

    try:
        res = run_bass_kernel_spmd(nc, in_maps, core_ids=list(range(N_CORES)))
    except Exception:
        if os.environ.get("BASS_TRACE"):
            # tracing infra failure — retry without profiling
            os.environ["BASS_NEVER_TRACE"] = "1"
            res = run_bass_kernel_spmd(nc, in_maps, core_ids=list(range(N_CORES)))
        else:
            raise
    LAST_RESULTS = res

    out = np.zeros((B, S, DIM), dtype=np.float32)
    for c in range(N_CORES):
        out[c // (N_CORES // B)] += res.results[c]["out_p"]
    out += b_out[None, None, :]
    out = out[:, inv_perm, :]    # un-permute rows
    return out
